# revision 17
# baseline (speedup 1.0000x reference)
"""Data-parallel 3x3 conv (NHWC 16x112x112x64, OHWI 64x3x3x64, pad=1, stride=1)
on 8 TRN2 NeuronCores via Bass/Tile.

v7 strategy (per core, 2 images) -- dense matmuls, fused input transpose,
per-image queue parallelism:
  - Host pre-casts x to bf16, pads each image to 128 rows (rows 1..112 =
    data, rows 0/113..127 = zeros), packs weights; device writes bf16 y
    (host upcasts). Error budget 2e-2 >> bf16 ~3e-3.
  - Input is transposed DIRECTLY from DRAM into T2[img][(pos2,c), j] via
    xbar DMA-transpose in full 128x128 tiles (j = 128*b + pr). Per-image
    tensors and queues: img0 on sync, img1 on scalar -- descriptor
    generation (~1.1ns/desc, serial per engine) and ring drain then run in
    parallel. Two queues transposing into ONE tensor race (v5 corruption),
    so destinations are separate tensors.
  - T2s[img] = partition-swapped/block-shifted copy of T2[img] (bulk
    SBUF->SBUF DMA on gpsimd): T2s[0:64,j]=T2[64:128,j-128] (O of prev col
    pair), T2s[64:128,j]=T2[0:64,j+128] (E of next). Turns the cross-block
    taps into dense K=128 matmuls: dense N=512 MMs sustain 216ns (2.4GHz)
    while K=64 tile_position pairs cap at ~1.6GHz.
  - Conv per 512-position chunk = 6 dense 128x128x512 matmuls in one PSUM
    bank: for dy in 0..2: mid (rhs=T2) + cross (rhs=T2s), rhs offset dy-1.
  - 9 warm-up matmuls during the lead-in flip the PE HAM clock gate to
    2.4GHz before real chunks start.
  - Vector evacuates PSUM f32 -> T3 bf16; output bands (32,20,4 blocks
    after chunks 7,12,13): xbar transpose T3 -> T4[img][pr,..] (img0 on
    scalar, img1 on sync -- opposite phase to input), then gpsimd DMA
    T4 -> bf16 NHWC DRAM.
"""
import sys

sys.path.insert(0, "/opt/trn_rl_repo")

import ml_dtypes
import numpy as np

import concourse.bass as bass
import concourse.tile as tile
from concourse import bacc, mybir
from concourse.bass_utils import run_bass_kernel_spmd

# Problem geometry (hardcoded per spec)
N, H, W, C = 16, 112, 112, 64
NCORES = 8
NPER = N // NCORES          # images per core
BLK = 128                   # free elems per column-pair block (2 cols x 64 ch)
NB = W // 2                 # 56 column-pair blocks per image
FPI = NB * BLK              # 7168 free elems per image
ZW = 256                    # zero gap width around each image
TLEN = ZW + FPI + ZW        # per-image T2/T2s length (7680)
CHUNK = 512                 # positions per psum chunk (4 blocks)
CHUNKS_IMG = FPI // CHUNK   # 14
NWARM = 10                  # PE warm-up matmuls during lead-in

IN_BANDS = [(0, 28), (28, 56)]
T2S_BANDS = [(0, 27), (27, 56)]          # band 2 emitted just-in-time
OUT_BANDS = ([(7, 0, 32), (12, 32, 52), (13, 52, 56)],        # img0
             [(3, 0, 16), (7, 16, 32), (11, 32, 48), (13, 48, 56)])  # img1

f16 = mybir.dt.bfloat16
f32 = mybir.dt.float32


def _conv_kernel(tc, x_ap, w_ap, y_ap):
    nc = tc.nc
    with tc.tile_pool(name="wp", bufs=1) as wp, \
         tc.tile_pool(name="big", bufs=1) as big, \
         tc.tile_pool(name="ps", bufs=7, space="PSUM") as psp, \
         tc.tile_pool(name="pw", bufs=1, space="PSUM") as pwp:

        wt = wp.tile([128, 1152], f16)  # [3 dy x (mid | crossE | crossO)]
        # gpsimd: keep plain DMAs off the transpose rings (mixing plain
        # and transpose jobs on one ring corrupts transposes)
        nc.gpsimd.dma_start(wt[:], w_ap)

        T2a = big.tile([128, TLEN], f16)
        T2b = big.tile([128, TLEN], f16)
        T3a = big.tile([128, FPI], f16)
        T3b = big.tile([128, FPI], f16)
        warm = big.tile([128, CHUNK], f16)
        T2 = (T2a, T2b)
        T3 = (T3a, T3b)
        # one T4 tile per output band: Tile's WAR tracking is tensor-
        # granular, so a shared T4 serializes xpose(k+1) behind out-DMA(k)
        T4 = {}
        for img in range(NPER):
            for bi, (_, b0, b1) in enumerate(OUT_BANDS[img]):
                t4band = big.tile([128, (b1 - b0) * BLK], f16)
                T4[(img, bi)] = t4band

        # gap regions (read by +-1 and +-BLK taps at image borders)
        for t in (T2a, T2b):
            nc.vector.memset(t[:, 0:ZW], 0)
            nc.vector.memset(t[:, TLEN - ZW:], 0)
        nc.vector.memset(warm[:], 0)

        xt = x_ap.tensor
        yt = y_ap.tensor
        s_row = W * C                     # DRAM row stride (elements)
        sx_img = 128 * W * C              # padded-x image stride
        s_img = H * W * C                 # y image stride

        # PE warm-up: accumulate ~3.4us of PE busy during the input lead-in
        # so the HAM clock gate flips to 2.4GHz before real chunks start
        pw = pwp.tile([128, CHUNK], f32)
        for _ in range(NWARM):
            nc.tensor.matmul(pw[:, :], wt[:, 0:128], warm[:],
                             start=True, stop=True, skip_group_check=True)

        # input: DRAM -> T2[img] fused xbar transposes, full 128-row tiles
        # from the host-padded x. img0's chain rides scalar (early ring),
        # img1's rides sync; T2s copies ride the same ring as their image's
        # transposes, interleaved so ring order == dependency order:
        #   scalar: wt, in0-b0, T2s0-band0, in0-b1, [JIT T2s0-band1], outx...
        #   sync:   in1-b0, T2s1-band0, in1-b1, T2s1-band1
        in_q = (nc.scalar, nc.sync)

        def emit_in_band(img, bi):
            b0, b1 = IN_BANDS[bi]
            t2v3 = T2[img][:].rearrange("p (a b) -> p a b", b=BLK)
            a0 = ZW // BLK
            dram = bass.AP(xt, img * sx_img + b0 * BLK,
                           [[s_row, 128], [1, (b1 - b0) * BLK]])
            in_q[img].dma_start(t2v3[:, a0 + b0: a0 + b1, :], dram,
                                transpose=True)

        for img in range(NPER):
            for bi in range(len(IN_BANDS)):
                emit_in_band(img, bi)

        # ---- compute: 6 dense matmuls + vector evac per chunk; output
        # bands per OUT_BANDS (out-xposes on scalar; out-DMA on gpsimd)
        out_q = (nc.scalar, nc.scalar)

        def emit_chunk(img, k):
            base = ZW + k * CHUNK
            f3 = k * CHUNK
            t2v = T2[img][:]
            ps = psp.tile([128, CHUNK], f32)
            for dy in range(3):
                off = base + dy - 1
                m = 384 * dy
                # mid: E,O of own col pair (2 taps each parity)
                nc.tensor.matmul(ps[:, :], wt[:, m: m + 128],
                                 t2v[:, off: off + CHUNK],
                                 start=(dy == 0), stop=False,
                                 skip_group_check=True)
                # crossE: O(b-1) -> even outs (zero-padded K half)
                nc.tensor.matmul(ps[:, :], wt[:, m + 128: m + 256],
                                 t2v[:, off - BLK: off - BLK + CHUNK],
                                 start=False, stop=False,
                                 skip_group_check=True)
                # crossO: E(b+1) -> odd outs (zero-padded K half)
                nc.tensor.matmul(ps[:, :], wt[:, m + 256: m + 384],
                                 t2v[:, off + BLK: off + BLK + CHUNK],
                                 start=False, stop=(dy == 2),
                                 skip_group_check=True)
            nc.vector.tensor_scalar_add(T3[img][:, f3: f3 + CHUNK], ps[:], 0.0)

        def emit_out_band(img, bi, b0, b1):
            nb = b1 - b0
            t4band = T4[(img, bi)]
            t4v3 = t4band[:].rearrange("p (a b) -> p a b", b=BLK)
            out_q[img].dma_start(
                t4v3[:, 0: nb, :],
                T3[img][:, b0 * BLK: b1 * BLK],
                transpose=True)
            dram = bass.AP(yt, img * s_img + b0 * BLK,
                           [[s_row, H], [1, nb * BLK]])
            nc.gpsimd.dma_start(dram, t4band[1:113, :])

        for img in range(NPER):
            bands = OUT_BANDS[img]
            bi = 0
            for k in range(CHUNKS_IMG):
                emit_chunk(img, k)
                while bi < len(bands) and bands[bi][0] == k:
                    _, b0, b1 = bands[bi]
                    emit_out_band(img, bi, b0, b1)
                    bi += 1


_CACHE = {}


def _build():
    if "nc" in _CACHE:
        return _CACHE["nc"]
    nc = bacc.Bacc("TRN2", target_bir_lowering=False, debug=False,
                   num_devices=NCORES)
    x_d = nc.dram_tensor("x", [NPER * 128 * W * C], f16, kind="ExternalInput").ap()
    w_d = nc.dram_tensor("w", [128, 1152], f16, kind="ExternalInput").ap()
    y_d = nc.dram_tensor("y", [NPER * H * W * C], f16, kind="ExternalOutput").ap()
    with tile.TileContext(nc) as tc:
        _conv_kernel(tc, x_d, w_d, y_d)
    nc.compile()
    _CACHE["nc"] = nc
    return nc


def _pack_weights(kernels):
    # kernels: (C_OUT=64, 3, 3, C_IN=64) f32, OHWI. wt[ci, dy, dx, co].
    wt = kernels.transpose(3, 1, 2, 0).astype(ml_dtypes.bfloat16)
    wpk = np.zeros((128, 1152), ml_dtypes.bfloat16)
    for dy in range(3):
        m = 384 * dy
        # mid: rhs = T2 at own block; M = [even out | odd out]
        wpk[0:64, m: m + 64] = wt[:, dy, 1]          # E -> even (dx=0)
        wpk[0:64, m + 64: m + 128] = wt[:, dy, 0]    # E -> odd  (dx=-1)
        wpk[64:128, m: m + 64] = wt[:, dy, 2]        # O -> even (dx=+1)
        wpk[64:128, m + 64: m + 128] = wt[:, dy, 1]  # O -> odd  (dx=0)
        # crossE: rhs = T2 at block b-1; only O half -> even outs
        c = m + 128
        wpk[64:128, c: c + 64] = wt[:, dy, 0]        # O(b-1) -> even (dx=-1)
        # crossO: rhs = T2 at block b+1; only E half -> odd outs
        c2 = m + 256
        wpk[0:64, c2 + 64: c2 + 128] = wt[:, dy, 2]  # E(b+1) -> odd (dx=+1)
    return wpk


def kernel(x, kernels, mode=None, _trace=False, **_):
    x = np.ascontiguousarray(np.asarray(x, dtype=np.float32))
    # pad each image to 128 rows: row 0 and rows 113..127 zero (conv pad
    # rows + clean xbar tiles)
    xb = np.zeros((N, 128, W, C), dtype=ml_dtypes.bfloat16)
    xb[:, 1:113] = x.astype(ml_dtypes.bfloat16)
    wpk = _pack_weights(np.asarray(kernels, dtype=np.float32))
    nc = _build()
    in_maps = [{"x": xb[i * NPER:(i + 1) * NPER].reshape(-1), "w": wpk}
               for i in range(NCORES)]
    res = run_bass_kernel_spmd(nc, in_maps, core_ids=list(range(NCORES)),
                               trace=_trace)
    out = np.concatenate(
        [np.asarray(res.results[i]["y"]).reshape(NPER, H, W, C)
         for i in range(NCORES)], axis=0)
    if _trace:
        kernel.last_result = res
    return out.astype(np.float32)


# revision 19
# speedup vs baseline: 1.0452x; 1.0452x over previous
"""Data-parallel 3x3 conv (NHWC 16x112x112x64, OHWI 64x3x3x64, pad=1, stride=1)
on 8 TRN2 NeuronCores via Bass/Tile.

v7 strategy (per core, 2 images) -- dense matmuls, fused input transpose,
per-image queue parallelism:
  - Host pre-casts x to bf16, pads each image to 128 rows (rows 1..112 =
    data, rows 0/113..127 = zeros), packs weights; device writes bf16 y
    (host upcasts). Error budget 2e-2 >> bf16 ~3e-3.
  - Input is transposed DIRECTLY from DRAM into T2[img][(pos2,c), j] via
    xbar DMA-transpose in full 128x128 tiles (j = 128*b + pr). Per-image
    tensors and queues: img0 on sync, img1 on scalar -- descriptor
    generation (~1.1ns/desc, serial per engine) and ring drain then run in
    parallel. Two queues transposing into ONE tensor race (v5 corruption),
    so destinations are separate tensors.
  - T2s[img] = partition-swapped/block-shifted copy of T2[img] (bulk
    SBUF->SBUF DMA on gpsimd): T2s[0:64,j]=T2[64:128,j-128] (O of prev col
    pair), T2s[64:128,j]=T2[0:64,j+128] (E of next). Turns the cross-block
    taps into dense K=128 matmuls: dense N=512 MMs sustain 216ns (2.4GHz)
    while K=64 tile_position pairs cap at ~1.6GHz.
  - Conv per 512-position chunk = 6 dense 128x128x512 matmuls in one PSUM
    bank: for dy in 0..2: mid (rhs=T2) + cross (rhs=T2s), rhs offset dy-1.
  - 9 warm-up matmuls during the lead-in flip the PE HAM clock gate to
    2.4GHz before real chunks start.
  - Vector evacuates PSUM f32 -> T3 bf16; output bands (32,20,4 blocks
    after chunks 7,12,13): xbar transpose T3 -> T4[img][pr,..] (img0 on
    scalar, img1 on sync -- opposite phase to input), then gpsimd DMA
    T4 -> bf16 NHWC DRAM.
"""
import sys

sys.path.insert(0, "/opt/trn_rl_repo")

import ml_dtypes
import numpy as np

import concourse.bass as bass
import concourse.tile as tile
from concourse import bacc, mybir
from concourse.bass_utils import run_bass_kernel_spmd

# Problem geometry (hardcoded per spec)
N, H, W, C = 16, 112, 112, 64
NCORES = 8
NPER = N // NCORES          # images per core
BLK = 128                   # free elems per column-pair block (2 cols x 64 ch)
NB = W // 2                 # 56 column-pair blocks per image
FPI = NB * BLK              # 7168 free elems per image
ZW = 256                    # zero gap width around each image
TLEN = ZW + FPI + ZW        # per-image T2/T2s length (7680)
CHUNK = 512                 # positions per psum chunk (4 blocks)
CHUNKS_IMG = FPI // CHUNK   # 14
NWARM = 15                  # PE warm-up matmuls during lead-in

IN_BANDS = [(0, 28), (28, 56)]
T2S_BANDS = [(0, 27), (27, 56)]          # band 2 emitted just-in-time
OUT_BANDS = ([(7, 0, 32), (12, 32, 52), (13, 52, 56)],        # img0
             [(3, 0, 16), (7, 16, 32), (11, 32, 48), (13, 48, 56)])  # img1

f16 = mybir.dt.bfloat16
f32 = mybir.dt.float32


def _conv_kernel(tc, x_ap, w_ap, y_ap):
    nc = tc.nc
    with tc.tile_pool(name="wp", bufs=1) as wp, \
         tc.tile_pool(name="big", bufs=1) as big, \
         tc.tile_pool(name="ps", bufs=7, space="PSUM") as psp, \
         tc.tile_pool(name="pw", bufs=1, space="PSUM") as pwp:

        wt = wp.tile([128, 1152], f16)  # [3 dy x (mid | crossE | crossO)]
        # gpsimd: keep plain DMAs off the transpose rings (mixing plain
        # and transpose jobs on one ring corrupts transposes)
        nc.gpsimd.dma_start(wt[:], w_ap)

        T2a = big.tile([128, TLEN], f16)
        T2b = big.tile([128, TLEN], f16)
        T3a = big.tile([128, FPI], f16)
        T3b = big.tile([128, FPI], f16)
        T4a = big.tile([128, FPI], f16)
        T4b = big.tile([128, FPI], f16)
        warm = big.tile([128, CHUNK], f16)
        T2 = (T2a, T2b)
        T3 = (T3a, T3b)
        T4 = (T4a, T4b)

        # gap regions (read by +-1 and +-BLK taps at image borders)
        for t in (T2a, T2b):
            nc.vector.memset(t[:, 0:ZW], 0)
            nc.vector.memset(t[:, TLEN - ZW:], 0)
        nc.vector.memset(warm[:], 0)

        xt = x_ap.tensor
        yt = y_ap.tensor
        s_row = W * C                     # DRAM row stride (elements)
        sx_img = 128 * W * C              # padded-x image stride
        s_img = H * W * C                 # y image stride

        # PE warm-up: accumulate ~3.4us of PE busy during the input lead-in
        # so the HAM clock gate flips to 2.4GHz before real chunks start
        pw = pwp.tile([128, CHUNK], f32)
        for _ in range(NWARM):
            nc.tensor.matmul(pw[:, :], wt[:, 0:128], warm[:],
                             start=True, stop=True, skip_group_check=True)

        # input: DRAM -> T2[img] fused xbar transposes, full 128-row tiles
        # from the host-padded x. img0's chain rides scalar (early ring),
        # img1's rides sync; T2s copies ride the same ring as their image's
        # transposes, interleaved so ring order == dependency order:
        #   scalar: wt, in0-b0, T2s0-band0, in0-b1, [JIT T2s0-band1], outx...
        #   sync:   in1-b0, T2s1-band0, in1-b1, T2s1-band1
        in_q = (nc.scalar, nc.sync)

        def emit_in_band(img, bi):
            b0, b1 = IN_BANDS[bi]
            t2v3 = T2[img][:].rearrange("p (a b) -> p a b", b=BLK)
            a0 = ZW // BLK
            dram = bass.AP(xt, img * sx_img + b0 * BLK,
                           [[s_row, 128], [1, (b1 - b0) * BLK]])
            in_q[img].dma_start(t2v3[:, a0 + b0: a0 + b1, :], dram,
                                transpose=True)

        for img in range(NPER):
            emit_in_band(img, 0)
        for img in range(NPER):
            emit_in_band(img, 1)

        # ---- compute: 9 dense matmuls + vector evac per chunk; output
        # bands per OUT_BANDS (out-xposes on scalar; out-DMA on gpsimd)

        def emit_chunk(img, k):
            base = ZW + k * CHUNK
            f3 = k * CHUNK
            t2v = T2[img][:]
            ps = psp.tile([128, CHUNK], f32)
            for dy in range(3):
                off = base + dy - 1
                m = 384 * dy
                # mid: E,O of own col pair (2 taps each parity)
                nc.tensor.matmul(ps[:, :], wt[:, m: m + 128],
                                 t2v[:, off: off + CHUNK],
                                 start=(dy == 0), stop=False,
                                 skip_group_check=True)
                # crossE: O(b-1) -> even outs (zero-padded K half)
                nc.tensor.matmul(ps[:, :], wt[:, m + 128: m + 256],
                                 t2v[:, off - BLK: off - BLK + CHUNK],
                                 start=False, stop=False,
                                 skip_group_check=True)
                # crossO: E(b+1) -> odd outs (zero-padded K half)
                nc.tensor.matmul(ps[:, :], wt[:, m + 256: m + 384],
                                 t2v[:, off + BLK: off + BLK + CHUNK],
                                 start=False, stop=(dy == 2),
                                 skip_group_check=True)
            nc.vector.tensor_scalar_add(T3[img][:, f3: f3 + CHUNK], ps[:], 0.0)

        def emit_out_band(img, bi, b0, b1):
            nb = b1 - b0
            t4v3 = T4[img][:].rearrange("p (a b) -> p a b", b=BLK)
            nc.scalar.dma_start(
                t4v3[:, b0: b0 + nb, :],
                T3[img][:, b0 * BLK: b1 * BLK],
                transpose=True)
            dram = bass.AP(yt, img * s_img + b0 * BLK,
                           [[s_row, H], [1, nb * BLK]])
            nc.gpsimd.dma_start(
                dram, T4[img][1:113, b0 * BLK: b1 * BLK])

        for img in range(NPER):
            bands = OUT_BANDS[img]
            bi = 0
            for k in range(CHUNKS_IMG):
                emit_chunk(img, k)
                while bi < len(bands) and bands[bi][0] == k:
                    _, b0, b1 = bands[bi]
                    emit_out_band(img, bi, b0, b1)
                    bi += 1


_CACHE = {}


def _build():
    if "nc" in _CACHE:
        return _CACHE["nc"]
    nc = bacc.Bacc("TRN2", target_bir_lowering=False, debug=False,
                   num_devices=NCORES)
    x_d = nc.dram_tensor("x", [NPER * 128 * W * C], f16, kind="ExternalInput").ap()
    w_d = nc.dram_tensor("w", [128, 1152], f16, kind="ExternalInput").ap()
    y_d = nc.dram_tensor("y", [NPER * H * W * C], f16, kind="ExternalOutput").ap()
    with tile.TileContext(nc) as tc:
        _conv_kernel(tc, x_d, w_d, y_d)
    nc.compile()
    _CACHE["nc"] = nc
    return nc


def _pack_weights(kernels):
    # kernels: (C_OUT=64, 3, 3, C_IN=64) f32, OHWI. wt[ci, dy, dx, co].
    wt = kernels.transpose(3, 1, 2, 0).astype(ml_dtypes.bfloat16)
    wpk = np.zeros((128, 1152), ml_dtypes.bfloat16)
    for dy in range(3):
        m = 384 * dy
        # mid: rhs = T2 at own block; M = [even out | odd out]
        wpk[0:64, m: m + 64] = wt[:, dy, 1]          # E -> even (dx=0)
        wpk[0:64, m + 64: m + 128] = wt[:, dy, 0]    # E -> odd  (dx=-1)
        wpk[64:128, m: m + 64] = wt[:, dy, 2]        # O -> even (dx=+1)
        wpk[64:128, m + 64: m + 128] = wt[:, dy, 1]  # O -> odd  (dx=0)
        # crossE: rhs = T2 at block b-1; only O half -> even outs
        c = m + 128
        wpk[64:128, c: c + 64] = wt[:, dy, 0]        # O(b-1) -> even (dx=-1)
        # crossO: rhs = T2 at block b+1; only E half -> odd outs
        c2 = m + 256
        wpk[0:64, c2 + 64: c2 + 128] = wt[:, dy, 2]  # E(b+1) -> odd (dx=+1)
    return wpk


def kernel(x, kernels, mode=None, _trace=False, **_):
    x = np.ascontiguousarray(np.asarray(x, dtype=np.float32))
    # pad each image to 128 rows: row 0 and rows 113..127 zero (conv pad
    # rows + clean xbar tiles)
    xb = np.zeros((N, 128, W, C), dtype=ml_dtypes.bfloat16)
    xb[:, 1:113] = x.astype(ml_dtypes.bfloat16)
    wpk = _pack_weights(np.asarray(kernels, dtype=np.float32))
    nc = _build()
    in_maps = [{"x": xb[i * NPER:(i + 1) * NPER].reshape(-1), "w": wpk}
               for i in range(NCORES)]
    res = run_bass_kernel_spmd(nc, in_maps, core_ids=list(range(NCORES)),
                               trace=_trace)
    out = np.concatenate(
        [np.asarray(res.results[i]["y"]).reshape(NPER, H, W, C)
         for i in range(NCORES)], axis=0)
    if _trace:
        kernel.last_result = res
    return out.astype(np.float32)


# revision 20
# speedup vs baseline: 1.0940x; 1.0467x over previous
"""Data-parallel 3x3 conv (NHWC 16x112x112x64, OHWI 64x3x3x64, pad=1, stride=1)
on 8 TRN2 NeuronCores via Bass/Tile.

v7 strategy (per core, 2 images) -- dense matmuls, fused input transpose,
per-image queue parallelism:
  - Host pre-casts x to bf16, pads each image to 128 rows (rows 1..112 =
    data, rows 0/113..127 = zeros), packs weights; device writes bf16 y
    (host upcasts). Error budget 2e-2 >> bf16 ~3e-3.
  - Input is transposed DIRECTLY from DRAM into T2[img][(pos2,c), j] via
    xbar DMA-transpose in full 128x128 tiles (j = 128*b + pr). Per-image
    tensors and queues: img0 on sync, img1 on scalar -- descriptor
    generation (~1.1ns/desc, serial per engine) and ring drain then run in
    parallel. Two queues transposing into ONE tensor race (v5 corruption),
    so destinations are separate tensors.
  - T2s[img] = partition-swapped/block-shifted copy of T2[img] (bulk
    SBUF->SBUF DMA on gpsimd): T2s[0:64,j]=T2[64:128,j-128] (O of prev col
    pair), T2s[64:128,j]=T2[0:64,j+128] (E of next). Turns the cross-block
    taps into dense K=128 matmuls: dense N=512 MMs sustain 216ns (2.4GHz)
    while K=64 tile_position pairs cap at ~1.6GHz.
  - Conv per 512-position chunk = 6 dense 128x128x512 matmuls in one PSUM
    bank: for dy in 0..2: mid (rhs=T2) + cross (rhs=T2s), rhs offset dy-1.
  - 9 warm-up matmuls during the lead-in flip the PE HAM clock gate to
    2.4GHz before real chunks start.
  - Vector evacuates PSUM f32 -> T3 bf16; output bands (32,20,4 blocks
    after chunks 7,12,13): xbar transpose T3 -> T4[img][pr,..] (img0 on
    scalar, img1 on sync -- opposite phase to input), then gpsimd DMA
    T4 -> bf16 NHWC DRAM.
"""
import sys

sys.path.insert(0, "/opt/trn_rl_repo")

import ml_dtypes
import numpy as np

import concourse.bass as bass
import concourse.tile as tile
from concourse import bacc, mybir
from concourse.bass_utils import run_bass_kernel_spmd

# Problem geometry (hardcoded per spec)
N, H, W, C = 16, 112, 112, 64
NCORES = 8
NPER = N // NCORES          # images per core
BLK = 128                   # free elems per column-pair block (2 cols x 64 ch)
NB = W // 2                 # 56 column-pair blocks per image
FPI = NB * BLK              # 7168 free elems per image
ZW = 256                    # zero gap width around each image
TLEN = ZW + FPI + ZW        # per-image T2/T2s length (7680)
CHUNK = 512                 # positions per psum chunk (4 blocks)
CHUNKS_IMG = FPI // CHUNK   # 14
NWARM = 15                  # PE warm-up matmuls during lead-in

IN_BANDS = [(0, 28), (28, 56)]
T2S_BANDS = [(0, 27), (27, 56)]          # band 2 emitted just-in-time
OUT_BANDS = ([(7, 0, 32), (12, 32, 52), (13, 52, 56)],        # img0
             [(3, 0, 16), (7, 16, 32), (11, 32, 48), (13, 48, 56)])  # img1

f16 = mybir.dt.bfloat16
f32 = mybir.dt.float32


def _conv_kernel(tc, x_ap, w_ap, y_ap):
    nc = tc.nc
    with tc.tile_pool(name="wp", bufs=1) as wp, \
         tc.tile_pool(name="big", bufs=1) as big, \
         tc.tile_pool(name="ps", bufs=7, space="PSUM") as psp, \
         tc.tile_pool(name="pw", bufs=1, space="PSUM") as pwp:

        wt = wp.tile([128, 1152], f16)  # [3 dy x (mid | crossE | crossO)]
        # gpsimd: keep plain DMAs off the transpose rings (mixing plain
        # and transpose jobs on one ring corrupts transposes)
        nc.gpsimd.dma_start(wt[:], w_ap)

        T2a = big.tile([128, TLEN], f16)
        T2b = big.tile([128, TLEN], f16)
        T3a = big.tile([128, FPI], f16)
        T3b = big.tile([128, FPI], f16)
        T4a = big.tile([128, FPI], f16)
        T4b = big.tile([128, FPI], f16)
        warm = big.tile([128, CHUNK], f16)
        T2 = (T2a, T2b)
        T3 = (T3a, T3b)
        T4 = (T4a, T4b)

        # gap regions (read by +-1 and +-BLK taps at image borders)
        for t in (T2a, T2b):
            nc.vector.memset(t[:, 0:ZW], 0)
            nc.vector.memset(t[:, TLEN - ZW:], 0)
        nc.vector.memset(warm[:], 0)

        xt = x_ap.tensor
        yt = y_ap.tensor
        s_row = W * C                     # DRAM row stride (elements)
        sx_img = 128 * W * C              # padded-x image stride
        s_img = H * W * C                 # y image stride

        # PE warm-up: accumulate ~3.4us of PE busy during the input lead-in
        # so the HAM clock gate flips to 2.4GHz before real chunks start
        pw = pwp.tile([128, CHUNK], f32)
        for _ in range(NWARM):
            nc.tensor.matmul(pw[:, :], wt[:, 0:128], warm[:],
                             start=True, stop=True, skip_group_check=True)

        # input: DRAM -> T2[img] fused xbar transposes, full 128-row tiles
        # from the host-padded x. img0's chain rides scalar (early ring),
        # img1's rides sync; T2s copies ride the same ring as their image's
        # transposes, interleaved so ring order == dependency order:
        #   scalar: wt, in0-b0, T2s0-band0, in0-b1, [JIT T2s0-band1], outx...
        #   sync:   in1-b0, T2s1-band0, in1-b1, T2s1-band1
        in_q = (nc.scalar, nc.sync)

        def emit_in_band(img, bi):
            b0, b1 = IN_BANDS[bi]
            t2v3 = T2[img][:].rearrange("p (a b) -> p a b", b=BLK)
            a0 = ZW // BLK
            dram = bass.AP(xt, img * sx_img + b0 * BLK,
                           [[s_row, 128], [1, (b1 - b0) * BLK]])
            in_q[img].dma_start(t2v3[:, a0 + b0: a0 + b1, :], dram,
                                transpose=True)

        for img in range(NPER):
            emit_in_band(img, 0)
        # band 1 of each image is emitted just-in-time inside the chunk
        # loop (before chunk 6, the first reader of block 28): chunks 0..5
        # then depend only on band 0 -- Tile's RAW deps are tensor-granular
        # last-writer-at-emission, so emitting band 1 up front would gate
        # chunk 0 on the whole image

        # ---- compute: 9 dense matmuls + vector evac per chunk; output
        # bands per OUT_BANDS (out-xposes on scalar; out-DMA on gpsimd)

        def emit_chunk(img, k):
            base = ZW + k * CHUNK
            f3 = k * CHUNK
            t2v = T2[img][:]
            ps = psp.tile([128, CHUNK], f32)
            for dy in range(3):
                off = base + dy - 1
                m = 384 * dy
                # mid: E,O of own col pair (2 taps each parity)
                nc.tensor.matmul(ps[:, :], wt[:, m: m + 128],
                                 t2v[:, off: off + CHUNK],
                                 start=(dy == 0), stop=False,
                                 skip_group_check=True)
                # crossE: O(b-1) -> even outs (zero-padded K half)
                nc.tensor.matmul(ps[:, :], wt[:, m + 128: m + 256],
                                 t2v[:, off - BLK: off - BLK + CHUNK],
                                 start=False, stop=False,
                                 skip_group_check=True)
                # crossO: E(b+1) -> odd outs (zero-padded K half)
                nc.tensor.matmul(ps[:, :], wt[:, m + 256: m + 384],
                                 t2v[:, off + BLK: off + BLK + CHUNK],
                                 start=False, stop=(dy == 2),
                                 skip_group_check=True)
            nc.vector.tensor_scalar_add(T3[img][:, f3: f3 + CHUNK], ps[:], 0.0)

        def emit_out_band(img, bi, b0, b1):
            nb = b1 - b0
            t4v3 = T4[img][:].rearrange("p (a b) -> p a b", b=BLK)
            nc.scalar.dma_start(
                t4v3[:, b0: b0 + nb, :],
                T3[img][:, b0 * BLK: b1 * BLK],
                transpose=True)
            dram = bass.AP(yt, img * s_img + b0 * BLK,
                           [[s_row, H], [1, nb * BLK]])
            nc.gpsimd.dma_start(
                dram, T4[img][1:113, b0 * BLK: b1 * BLK])

        for img in range(NPER):
            bands = OUT_BANDS[img]
            bi = 0
            for k in range(CHUNKS_IMG):
                if k == 5:
                    emit_in_band(img, 1)
                emit_chunk(img, k)
                while bi < len(bands) and bands[bi][0] == k:
                    _, b0, b1 = bands[bi]
                    emit_out_band(img, bi, b0, b1)
                    bi += 1


_CACHE = {}


def _build():
    if "nc" in _CACHE:
        return _CACHE["nc"]
    nc = bacc.Bacc("TRN2", target_bir_lowering=False, debug=False,
                   num_devices=NCORES)
    x_d = nc.dram_tensor("x", [NPER * 128 * W * C], f16, kind="ExternalInput").ap()
    w_d = nc.dram_tensor("w", [128, 1152], f16, kind="ExternalInput").ap()
    y_d = nc.dram_tensor("y", [NPER * H * W * C], f16, kind="ExternalOutput").ap()
    with tile.TileContext(nc) as tc:
        _conv_kernel(tc, x_d, w_d, y_d)
    nc.compile()
    _CACHE["nc"] = nc
    return nc


def _pack_weights(kernels):
    # kernels: (C_OUT=64, 3, 3, C_IN=64) f32, OHWI. wt[ci, dy, dx, co].
    wt = kernels.transpose(3, 1, 2, 0).astype(ml_dtypes.bfloat16)
    wpk = np.zeros((128, 1152), ml_dtypes.bfloat16)
    for dy in range(3):
        m = 384 * dy
        # mid: rhs = T2 at own block; M = [even out | odd out]
        wpk[0:64, m: m + 64] = wt[:, dy, 1]          # E -> even (dx=0)
        wpk[0:64, m + 64: m + 128] = wt[:, dy, 0]    # E -> odd  (dx=-1)
        wpk[64:128, m: m + 64] = wt[:, dy, 2]        # O -> even (dx=+1)
        wpk[64:128, m + 64: m + 128] = wt[:, dy, 1]  # O -> odd  (dx=0)
        # crossE: rhs = T2 at block b-1; only O half -> even outs
        c = m + 128
        wpk[64:128, c: c + 64] = wt[:, dy, 0]        # O(b-1) -> even (dx=-1)
        # crossO: rhs = T2 at block b+1; only E half -> odd outs
        c2 = m + 256
        wpk[0:64, c2 + 64: c2 + 128] = wt[:, dy, 2]  # E(b+1) -> odd (dx=+1)
    return wpk


def kernel(x, kernels, mode=None, _trace=False, **_):
    x = np.ascontiguousarray(np.asarray(x, dtype=np.float32))
    # pad each image to 128 rows: row 0 and rows 113..127 zero (conv pad
    # rows + clean xbar tiles)
    xb = np.zeros((N, 128, W, C), dtype=ml_dtypes.bfloat16)
    xb[:, 1:113] = x.astype(ml_dtypes.bfloat16)
    wpk = _pack_weights(np.asarray(kernels, dtype=np.float32))
    nc = _build()
    in_maps = [{"x": xb[i * NPER:(i + 1) * NPER].reshape(-1), "w": wpk}
               for i in range(NCORES)]
    res = run_bass_kernel_spmd(nc, in_maps, core_ids=list(range(NCORES)),
                               trace=_trace)
    out = np.concatenate(
        [np.asarray(res.results[i]["y"]).reshape(NPER, H, W, C)
         for i in range(NCORES)], axis=0)
    if _trace:
        kernel.last_result = res
    return out.astype(np.float32)
